# revision 14
# baseline (speedup 1.0000x reference)
"""Trainium2 Bass kernel for causal multi-head attention with RoPE.

Problem: B=2, S=2048, D=2048, H=16 heads of HD=128.
  q/k/v = x @ w{q,k,v}.T ; RoPE(q,k) ; causal softmax(q k^T/sqrt(HD)) @ v ; @ wo.T

Sharding (8 cores): batch (2) x head-group (4 groups of 4 heads).
Each core: full projections for its 4 heads on its batch, attention, and a
partial output projection (row-shard of wo). Host sums the 4 partials per batch.

Device-side layout tricks (all host-prepared, free at HW time):
  - x is passed transposed (xT [D,S]) so QKV matmuls need no on-chip transpose.
  - scores are computed transposed [k,q]: softmax sums via ones-matmul, AV
    yields oT [d,q] whose 128-col slices are exactly the out-proj lhsT.
  - RoPE pairs are pre-permuted into rotate-half layout (even dims in
    partitions 0:64, odd in 64:128) by permuting wq/wk rows on the host.
  - causal masking: upper-triangle k-tiles are skipped entirely; the 4
    diagonal-crossing tile shapes use precomputed 0/1 multiplicative masks.
"""

import sys

sys.path.insert(0, "/opt/trn_rl_repo")

from contextlib import ExitStack

import numpy as np
import ml_dtypes

import concourse.bass as bass
import concourse.tile as tile
from concourse import bacc, mybir
from concourse.bass_utils import run_bass_kernel_spmd

B, S, D, H = 2, 2048, 2048, 16
HD, HD2 = 128, 64
NCORES = 8
HPC = 4              # heads per core
DPC = HPC * HD       # 512
GROUPS = H // HPC    # 4 head-groups (x 2 batches = 8 cores)
SCALE = 1.0 / float(np.sqrt(HD))

ST = 512             # q-tile width (free dim of most matmuls)
NST = S // ST        # 4
KT = 128             # k-tile height (partition dim of score tiles)
NKT = S // KT        # 16
NC_CHUNK = D // 128  # 16 contraction chunks for projections

BF16 = mybir.dt.bfloat16
FP16 = mybir.dt.float16
F32 = mybir.dt.float32
NPBF16 = ml_dtypes.bfloat16
NPFP16 = np.float16
LN_INV512 = float(np.log(1.0 / 512.0))  # exp bias so fp16 sums can't overflow

EXP_FN = mybir.ActivationFunctionType.Exp


def build_program(mode: str):
    """mode: 'causal' (skip upper tiles, diag masks), 'dense' (no mask),
    'masked' (multiply every exp tile by a streamed exp(mask) tile)."""
    assert mode in ("causal", "dense", "masked")
    nc = bacc.Bacc(
        "TRN2",
        target_bir_lowering=False,
        debug=False,
        enable_asserts=False,
        num_devices=NCORES,
    )
    xT = nc.dram_tensor("xT", [D, S], BF16, kind="ExternalInput").ap()
    wqT = nc.dram_tensor("wqT", [D, DPC], BF16, kind="ExternalInput").ap()
    wkT = nc.dram_tensor("wkT", [D, DPC], BF16, kind="ExternalInput").ap()
    wvT = nc.dram_tensor("wvT", [D, DPC], BF16, kind="ExternalInput").ap()
    woT = nc.dram_tensor("woT", [DPC, D], BF16, kind="ExternalInput").ap()
    cosT = nc.dram_tensor("cosT", [HD, S], BF16, kind="ExternalInput").ap()
    sinT = nc.dram_tensor("sinT", [HD, S], BF16, kind="ExternalInput").ap()
    mask4 = emask = None
    if mode == "causal":
        mask4 = nc.dram_tensor("mask4", [4, KT, ST], FP16, kind="ExternalInput").ap()
    if mode == "masked":
        emask = nc.dram_tensor("emask", [S, S], FP16, kind="ExternalInput").ap()
    out = nc.dram_tensor("out", [S, D], F32, kind="ExternalOutput").ap()

    with tile.TileContext(nc) as tc, ExitStack() as ctx:
        _body(ctx, tc, mode, xT, wqT, wkT, wvT, woT, cosT, sinT, mask4, emask, out)
    nc.compile()
    return nc


def _body(ctx, tc, mode, xT, wqT, wkT, wvT, woT, cosT, sinT, mask4, emask, out):
    nc = tc.nc
    resid = ctx.enter_context(tc.tile_pool(name="resid", bufs=1))
    xpool = ctx.enter_context(tc.tile_pool(name="xpool", bufs=2))
    psum = ctx.enter_context(tc.tile_pool(name="psum", bufs=1, space="PSUM"))
    tmp = ctx.enter_context(tc.tile_pool(name="tmp", bufs=1))

    # ---- resident weights / constants ----
    # Loads are split per 128-row chunk so they spread across DMA queues and
    # the first Q/K matmuls can start as soon as their chunks land.
    wq_sb = resid.tile([128, NC_CHUNK, DPC], BF16, name="wq_sb")
    wk_sb = resid.tile([128, NC_CHUNK, DPC], BF16, name="wk_sb")
    wv_sb = resid.tile([128, NC_CHUNK, DPC], BF16, name="wv_sb")
    wo_sb = resid.tile([128, HPC, D], BF16, name="wo_sb")
    wqT_c = wqT.rearrange("(c p) m -> c p m", p=128)
    wkT_c = wkT.rearrange("(c p) m -> c p m", p=128)
    wvT_c = wvT.rearrange("(c p) m -> c p m", p=128)
    for c in range(NC_CHUNK):
        nc.scalar.dma_start(out=wq_sb[:, c, :], in_=wqT_c[c])
        nc.scalar.dma_start(out=wk_sb[:, c, :], in_=wkT_c[c])
    for c in range(NC_CHUNK):
        nc.scalar.dma_start(out=wv_sb[:, c, :], in_=wvT_c[c])
    woT_c = woT.rearrange("(h p) n -> h p n", p=128)
    for hh in range(HPC):
        nc.scalar.dma_start(out=wo_sb[:, hh, :], in_=woT_c[hh])
    cos_sb = resid.tile([128, S], BF16, name="cos_sb")
    nc.scalar.dma_start(out=cos_sb, in_=cosT)
    sin_sb = resid.tile([128, S], BF16, name="sin_sb")
    nc.scalar.dma_start(out=sin_sb, in_=sinT)
    mask_sb = None
    if mode == "causal":
        mask_sb = resid.tile([128, 4, ST], FP16, name="mask_sb")
        nc.scalar.dma_start(out=mask_sb, in_=mask4.rearrange("g p m -> p g m"))

    ones_sb = resid.tile([128, 1], FP16, name="ones_sb")
    nc.vector.memset(ones_sb, 1.0)
    ebias_sb = resid.tile([128, 1], F32, name="ebias_sb")
    nc.vector.memset(ebias_sb, LN_INV512)

    # ---- resident activations ----
    qT_sb = resid.tile([128, HPC, S], BF16, name="qT_sb")   # [d, h, q-pos]
    kT_sb = resid.tile([128, HPC, S], BF16, name="kT_sb")   # [d, h, k-pos]
    v_sb = resid.tile([128, NKT, DPC], FP16, name="v_sb")   # [k-pos%128, k-tile, hd]
    oT_sb = resid.tile([128, HPC, S], BF16, name="oT_sb")   # [d, h, q-pos]

    # ================= Phase 1: QKV projections + RoPE =================
    # s-tiles processed in pairs so one weight load feeds two matmuls.
    def rope(ps, dstT, h, ssl):
        stg = tmp.tile([128, ST], BF16, tag="stg", bufs=3, name="stg")
        nc.scalar.copy(stg, ps)
        swp = tmp.tile([128, ST], BF16, tag="swp", bufs=3, name="swp")
        nc.sync.dma_start(out=swp[0:64, :], in_=stg[64:128, :])
        nc.sync.dma_start(out=swp[64:128, :], in_=stg[0:64, :])
        t1 = tmp.tile([128, ST], BF16, tag="t1", bufs=2, name="t1")
        t2 = tmp.tile([128, ST], BF16, tag="t2", bufs=2, name="t2")
        nc.vector.tensor_mul(t1, stg, cos_sb[:, ssl])
        nc.vector.tensor_mul(t2, swp, sin_sb[:, ssl])
        nc.vector.tensor_add(dstT[:, h, ssl], t1, t2)

    for stp in range(NST // 2):
        sta, stb = 2 * stp, 2 * stp + 1
        ssla = slice(sta * ST, (sta + 1) * ST)
        sslb = slice(stb * ST, (stb + 1) * ST)
        xa = xpool.tile([128, NC_CHUNK, ST], BF16, tag="x", bufs=2, name="xa")
        xb = xpool.tile([128, NC_CHUNK, ST], BF16, tag="x", bufs=2, name="xb")
        xTa = xT[:, ssla].rearrange("(c p) s -> c p s", p=128)
        xTb = xT[:, sslb].rearrange("(c p) s -> c p s", p=128)
        for c in range(NC_CHUNK):
            nc.sync.dma_start(out=xa[:, c, :], in_=xTa[c])
            nc.sync.dma_start(out=xb[:, c, :], in_=xTb[c])

        for h in range(HPC):
            pqa = psum.tile([128, ST], F32, tag="pj", bufs=5, name="ps_qa")
            pqb = psum.tile([128, ST], F32, tag="pj", bufs=5, name="ps_qb")
            pka = psum.tile([128, ST], F32, tag="pj", bufs=5, name="ps_ka")
            pkb = psum.tile([128, ST], F32, tag="pj", bufs=5, name="ps_kb")
            for c in range(NC_CHUNK):
                wq_c = wq_sb[:, c, h * HD:(h + 1) * HD]
                wk_c = wk_sb[:, c, h * HD:(h + 1) * HD]
                st_ = (c == 0)
                sp_ = (c == NC_CHUNK - 1)
                nc.tensor.matmul(pqa, wq_c, xa[:, c, :], start=st_, stop=sp_)
                nc.tensor.matmul(pqb, wq_c, xb[:, c, :], start=st_, stop=sp_)
                nc.tensor.matmul(pka, wk_c, xa[:, c, :], start=st_, stop=sp_)
                nc.tensor.matmul(pkb, wk_c, xb[:, c, :], start=st_, stop=sp_)
            rope(pqa, qT_sb, h, ssla)
            rope(pqb, qT_sb, h, sslb)
            rope(pka, kT_sb, h, ssla)
            rope(pkb, kT_sb, h, sslb)

        # V projection: x-chunk stationary -> natural [s, hd] orientation
        for x_sb, st in ((xa, sta), (xb, stb)):
            for s4 in range(ST // 128):
                stile = st * (ST // 128) + s4
                pv = psum.tile([128, DPC], F32, tag="pv", bufs=3, name="ps_pv")
                for c in range(NC_CHUNK):
                    nc.tensor.matmul(
                        pv,
                        x_sb[:, c, s4 * 128:(s4 + 1) * 128],
                        wv_sb[:, c, :],
                        start=(c == 0),
                        stop=(c == NC_CHUNK - 1),
                    )
                nc.scalar.copy(v_sb[:, stile, :], pv)

    # ================= Phase 2: attention per head =================
    # Normalization of (h, qt) is emitted after the NEXT (h, qt)'s k-loop so
    # its recip->broadcast->mul chain (DVE/GpSimd only) hides under PE work.
    def emit_norm(h, qt, po, psm):
        qsl = slice(qt * ST, (qt + 1) * ST)
        r_row = tmp.tile([1, ST], F32, tag="r", bufs=2, name="r_row")
        nc.vector.reciprocal(r_row, psm)
        rb_sb = tmp.tile([128, ST], F32, tag="rb", bufs=2, name="rb_sb")
        nc.gpsimd.partition_broadcast(rb_sb, r_row)
        nc.vector.tensor_mul(oT_sb[:, h, qsl], po, rb_sb)

    # Output projection for one q-tile's 4 row-blocks (needs that qt's norms).
    def emit_outproj(qt):
        for s128 in range(qt * 4, (qt + 1) * 4):
            pouts = []
            for j in range(NST):
                pj_ = psum.tile([128, ST], F32, tag="pj", bufs=5, name="ps_out")
                pouts.append(pj_)
            for h in range(HPC):
                for j in range(NST):
                    nc.tensor.matmul(
                        pouts[j],
                        oT_sb[:, h, s128 * 128:(s128 + 1) * 128],
                        wo_sb[:, h, j * ST:(j + 1) * ST],
                        start=(h == 0),
                        stop=(h == HPC - 1),
                    )
            for j in range(NST):
                o_sb = tmp.tile([128, ST], F32, tag="osb", bufs=4, name="o_sb")
                nc.vector.tensor_copy(o_sb, pouts[j])
                nc.sync.dma_start(
                    out=out[s128 * 128:(s128 + 1) * 128, j * ST:(j + 1) * ST],
                    in_=o_sb,
                )

    pending = None
    for qt in range(NST):
        for h in range(HPC):
            nkt = 4 * (qt + 1) if mode == "causal" else NKT
            po = psum.tile([128, ST], F32, tag="pj", bufs=5, name="ps_po")
            acc = tmp.tile([128, ST], FP16, tag="acc", bufs=2, name="acc")
            for kt in range(nkt):
                di = kt - 4 * qt
                # diagonal tiles only have valid queries at columns >= di*KT
                q0 = di * KT if (mode == "causal" and di >= 0) else 0
                qsl = slice(qt * ST + q0, (qt + 1) * ST)
                pss = psum.tile([128, ST], F32, tag="pv", bufs=3, name="ps_s")
                nc.tensor.matmul(
                    pss[:, q0:],
                    kT_sb[:, h, kt * KT:(kt + 1) * KT],
                    qT_sb[:, h, qsl],
                    start=True,
                    stop=True,
                )
                e_sb = tmp.tile([128, ST], FP16, tag="e", bufs=4, name="e_sb")
                nc.scalar.activation(
                    e_sb[:, q0:], pss[:, q0:], func=EXP_FN,
                    scale=SCALE, bias=ebias_sb,
                )
                if mode == "causal":
                    if di >= 0:
                        nc.vector.tensor_mul(
                            e_sb[:, q0:], e_sb[:, q0:], mask_sb[:, di, q0:]
                        )
                elif mode == "masked":
                    m_sb = tmp.tile([128, ST], FP16, tag="m", bufs=4, name="m_sb")
                    nc.sync.dma_start(
                        out=m_sb, in_=emask[kt * KT:(kt + 1) * KT, qsl]
                    )
                    nc.vector.tensor_mul(e_sb, e_sb, m_sb)
                if kt == 0:
                    nc.vector.tensor_copy(acc, e_sb)
                else:
                    nc.vector.tensor_add(
                        acc[:, q0:], acc[:, q0:], e_sb[:, q0:]
                    )
                nc.tensor.matmul(
                    po[:, q0:],
                    v_sb[:, kt, h * HD:(h + 1) * HD],
                    e_sb[:, q0:],
                    start=(kt == 0), stop=(kt == nkt - 1),
                )
            psm = psum.tile([1, ST], F32, tag="pj", bufs=5, name="ps_sum")
            nc.tensor.matmul(psm, ones_sb, acc, start=True, stop=True)
            if pending is not None:
                emit_norm(*pending)
            pending = (h, qt, po, psm)
        if qt >= 1:
            emit_outproj(qt - 1)
    emit_norm(*pending)
    emit_outproj(NST - 1)


# ---------------------------------------------------------------------------
# Host side
# ---------------------------------------------------------------------------

_PROGRAMS: dict = {}


def _get_program(mode: str):
    if mode not in _PROGRAMS:
        _PROGRAMS[mode] = build_program(mode)
    return _PROGRAMS[mode]


_PERM = np.concatenate([np.arange(0, HD, 2), np.arange(1, HD, 2)])  # rotate-half


def _mask4_np() -> np.ndarray:
    m = np.zeros((4, KT, ST), dtype=np.float32)
    p = np.arange(KT)[:, None]
    qf = np.arange(ST)[None, :]
    for di in range(4):
        m[di] = (qf >= di * KT + p).astype(np.float32)
    return m.astype(NPFP16)


def _classify_mask(m: np.ndarray) -> str:
    if not np.any(m):
        return "dense"
    causal = np.triu(np.full((S, S), -1e9, dtype=np.float32), 1)
    if np.array_equal(m, causal):
        return "causal"
    return "masked"


def make_in_maps(x, freqs_cos, freqs_sin, mask, wq, wk, wv, wo, mode):
    """Build the 8 per-core input dicts."""
    cosT = np.ascontiguousarray(np.asarray(freqs_cos, np.float32).T)  # [64, S]
    sinT = np.ascontiguousarray(np.asarray(freqs_sin, np.float32).T)
    cosT2 = np.concatenate([cosT, cosT], 0).astype(NPBF16)            # [128, S]
    # rows 0:64 get -sin (dst_e = qe*c - qo*s), rows 64:128 get +sin
    sinT2 = np.concatenate([-sinT, sinT], 0).astype(NPBF16)
    mask4 = _mask4_np() if mode == "causal" else None
    em = None
    if mode == "masked":
        # kernel indexes emask as [k, q]; mask is [q, k]
        em = np.exp(np.asarray(mask, np.float32).reshape(S, S)).T
        em = np.ascontiguousarray(em).astype(NPFP16)

    # permuted rows (within each head) for wq / wk
    perm_rows = (np.arange(H)[:, None] * HD + _PERM[None, :]).reshape(-1)
    wq_p = np.asarray(wq, np.float32)[perm_rows]
    wk_p = np.asarray(wk, np.float32)[perm_rows]
    wv_f = np.asarray(wv, np.float32)
    wo_f = np.asarray(wo, np.float32)
    x_f = np.asarray(x, np.float32)

    in_maps = []
    for core in range(NCORES):
        b, g = divmod(core, GROUPS)
        rs = slice(g * DPC, (g + 1) * DPC)
        im = {
            "xT": np.ascontiguousarray(x_f[b].T).astype(NPBF16),
            "wqT": np.ascontiguousarray(wq_p[rs].T).astype(NPBF16),
            "wkT": np.ascontiguousarray(wk_p[rs].T).astype(NPBF16),
            "wvT": np.ascontiguousarray(wv_f[rs].T).astype(NPBF16),
            "woT": np.ascontiguousarray(wo_f[:, rs].T).astype(NPBF16),
            "cosT": cosT2,
            "sinT": sinT2,
        }
        if mode == "causal":
            im["mask4"] = mask4
        if mode == "masked":
            im["emask"] = em
        in_maps.append(im)
    return in_maps


def assemble(results) -> np.ndarray:
    out = np.zeros((B, S, D), dtype=np.float32)
    for core in range(NCORES):
        b = core // GROUPS
        out[b] += results[core]["out"]
    return out


def kernel(x, freqs_cos, freqs_sin, mask, wq, wk, wv, wo, **run_kwargs):
    mode = _classify_mask(np.asarray(mask, np.float32).reshape(S, S))
    nc = _get_program(mode)
    in_maps = make_in_maps(x, freqs_cos, freqs_sin, mask, wq, wk, wv, wo, mode)
    res = run_bass_kernel_spmd(nc, in_maps, core_ids=list(range(NCORES)), **run_kwargs)
    out = assemble(res.results)
    kernel.last_results = res
    return out
